# revision 2
# baseline (speedup 1.0000x reference)
"""Trainium2 Bass kernel for batch-8 multi-head attention.

Strategy: pure data parallelism — one batch element per NeuronCore (B=8,
8 cores), zero collectives.  All inputs are pre-arranged on the host so the
device kernel only ever runs dense matmuls in its preferred layouts:

  per-core DRAM inputs (bf16 unless noted):
    xT     [768, 1024]   x[b].T                    (feature-major activations)
    w_qk   [768, 1536]   [W_q * SCALE | W_k]       (stationary for QK^T)
    w_v    [768, 768]    W_v                       (moving for V)
    w_proj [768, 768]    W_proj                    (stationary for proj)
    b_all  [128, 18] f32 per-partition bias chunks (12 qk + 6 proj)
    b_v    [128, 12, 64] f32  V bias broadcast along partitions
  output:
    outT   [768, 1024] f32  (x[b] @ ... final)^T — host transposes back

Device pipeline per core:
  QK^T = w_qk^T @ xT                 -> 12 tiles [128, 1024], 2 heads/tile
  V    = xT^T @ w_v + b_v            -> 8 tiles [128, 12, 65], ones col fused
  per head h (packed 2/tile, concurrent PE sub-array matmuls):
    S^T[m]   = K_h @ Q_h^T           (K=64 contraction)
    expS     = exp(S^T)  on ScalarE  (no max subtraction: |logits| < ~8)
    [O^T|s]  = [V_m|1]^T @ expS      (PSUM accumulate over m; row 64 = sums)
    oT       = O^T * (1/s)           (reciprocal + partition-broadcast + mul)
  outT = w_proj^T @ oT + b_proj
"""

import os
import sys

os.environ.setdefault("BASS_PERFETTO_PROFILE_ALL_CORES", "1")
if "/opt/trn_rl_repo" not in sys.path:
    sys.path.insert(0, "/opt/trn_rl_repo")

import numpy as np
import ml_dtypes

B, N, C, H = 8, 1024, 768, 12
D = C // H                # 64 head dim
SCALE = D ** -0.5
NCORES = 8
KT = C // 128             # 6 contraction tiles over C
MT = N // 128             # 8 token blocks
NJ = N // 512             # 2 query chunks of 512
BF16 = ml_dtypes.bfloat16

_CACHE = {}


def build_nc():
    """Build + compile the per-core Bass graph (identical on all 8 cores)."""
    import concourse.tile as tile
    from concourse import bacc, mybir

    f32 = mybir.dt.float32
    bf16 = mybir.dt.bfloat16
    Exp = mybir.ActivationFunctionType.Exp

    nc = bacc.Bacc("TRN2", target_bir_lowering=False, debug=False,
                   num_devices=NCORES)

    xT_e = nc.dram_tensor("xT", [C, N], bf16, kind="ExternalInput").ap()
    wqk_e = nc.dram_tensor("w_qk", [C, 2 * C], bf16, kind="ExternalInput").ap()
    wv_e = nc.dram_tensor("w_v", [C, C], bf16, kind="ExternalInput").ap()
    wp_e = nc.dram_tensor("w_proj", [C, C], bf16, kind="ExternalInput").ap()
    ball_e = nc.dram_tensor("b_all", [128, 18], f32, kind="ExternalInput").ap()
    bv_e = nc.dram_tensor("b_v", [128, H, D], f32, kind="ExternalInput").ap()
    out_e = nc.dram_tensor("outT", [C, N], f32, kind="ExternalOutput").ap()

    with tile.TileContext(nc) as tc:
        from contextlib import ExitStack

        with ExitStack() as es:
            persist = es.enter_context(tc.tile_pool(name="persist", bufs=1))

            # ---- load inputs --------------------------------------------
            xT = [persist.tile([128, N], bf16, name=f"xT{k}", tag=f"xT{k}") for k in range(KT)]
            wqk = [persist.tile([128, 2 * C], bf16, name=f"wqk{k}", tag=f"wqk{k}") for k in range(KT)]
            wv = [persist.tile([128, C], bf16, name=f"wv{k}", tag=f"wv{k}") for k in range(KT)]
            wp = [persist.tile([128, C], bf16, name=f"wp{k}", tag=f"wp{k}") for k in range(KT)]
            ball = persist.tile([128, 18], f32, name="ball", tag="ball")
            bv = persist.tile([128, H, D], f32, name="bv", tag="bv")
            for k in range(KT):
                sl = slice(128 * k, 128 * (k + 1))
                nc.sync.dma_start(xT[k][:], xT_e[sl, :])
                nc.sync.dma_start(wqk[k][:], wqk_e[sl, :])
                nc.sync.dma_start(wv[k][:], wv_e[sl, :])
                nc.sync.dma_start(wp[k][:], wp_e[sl, :])
            nc.sync.dma_start(ball[:], ball_e[:])
            nc.sync.dma_start(bv[:], bv_e[:])

            # persistent intermediates
            qkT = [persist.tile([128, N], bf16, name=f"qkT{m}", tag=f"qkT{m}") for m in range(12)]
            v_sb = [persist.tile([128, H, D + 1], bf16, name=f"v{t}", tag=f"v{t}") for t in range(MT)]
            oT = [persist.tile([128, N], bf16, name=f"oT{m}", tag=f"oT{m}") for m in range(KT)]

            # ---- phase 1a: QK^T = w_qk^T @ xT (+bias) -------------------
            # order Q/K chunks pairwise so head-pairs finish early
            with tc.tile_pool(name="qk_ps", bufs=2, space="PSUM") as qk_pool:
                for m in [0, 6, 1, 7, 2, 8, 3, 9, 4, 10, 5, 11]:
                    ps = qk_pool.tile([128, N], f32, name="qk_ps", tag="qk_ps")
                    for j in range(NJ):
                        for k in range(KT):
                            nc.tensor.matmul(
                                ps[:, 512 * j:512 * (j + 1)],
                                lhsT=wqk[k][:, 128 * m:128 * (m + 1)],
                                rhs=xT[k][:, 512 * j:512 * (j + 1)],
                                start=(k == 0), stop=(k == KT - 1),
                            )
                    nc.vector.tensor_scalar_add(qkT[m][:], ps[:], ball[:, m:m + 1])

                # ---- phase 1b: V = xT^T @ w_v + b_v, ones col fused -----
                with tc.tile_pool(name="v_ps", bufs=2, space="PSUM") as v_pool:
                    for t in range(MT):
                        ps = v_pool.tile([128, C], f32, name="v_ps", tag="v_ps")
                        for c0, cw in ((0, 512), (512, 256)):
                            for k in range(KT):
                                nc.tensor.matmul(
                                    ps[:, c0:c0 + cw],
                                    lhsT=xT[k][:, 128 * t:128 * (t + 1)],
                                    rhs=wv[k][:, c0:c0 + cw],
                                    start=(k == 0), stop=(k == KT - 1),
                                )
                        nc.gpsimd.memset(v_sb[t][:, :, D:D + 1], 1.0)
                        nc.vector.tensor_add(
                            v_sb[t][:, :, 0:D],
                            ps[:].rearrange("p (h x) -> p h x", x=D),
                            bv[:],
                        )

            # ---- phase 2: attention per head ----------------------------
            with ExitStack() as es2:
                s_pool = es2.enter_context(
                    tc.tile_pool(name="s_ps", bufs=2, space="PSUM"))
                o_pool = es2.enter_context(
                    tc.tile_pool(name="o_ps", bufs=4, space="PSUM"))
                e_pool = es2.enter_context(tc.tile_pool(name="expS", bufs=3))
                r_pool = es2.enter_context(tc.tile_pool(name="recip", bufs=2))
                rb_pool = es2.enter_context(tc.tile_pool(name="recipb", bufs=2))

                for h in range(H):
                    qt = qkT[h // 2]
                    kt = qkT[6 + h // 2]
                    po = 64 * (h % 2)
                    o_ps = [o_pool.tile([65, 512], f32, name="o_ps", tag="o_ps") for _ in range(NJ)]
                    for m in range(MT):
                        s_ps = s_pool.tile([128, N], f32, name="s_ps", tag="s_ps")
                        for j in range(NJ):
                            nc.tensor.matmul(
                                s_ps[:, 512 * j:512 * (j + 1)],
                                lhsT=kt[po:po + 64, 128 * m:128 * (m + 1)],
                                rhs=qt[po:po + 64, 512 * j:512 * (j + 1)],
                                start=True, stop=True,
                            )
                        e_sb = e_pool.tile([128, N], bf16, name="e_sb", tag="e_sb")
                        nc.scalar.activation(e_sb[:], s_ps[:], Exp)
                        for j in range(NJ):
                            nc.tensor.matmul(
                                o_ps[j][:, :],
                                lhsT=v_sb[m][:, h, :],
                                rhs=e_sb[:, 512 * j:512 * (j + 1)],
                                start=(m == 0), stop=(m == MT - 1),
                            )
                    # normalize: oT[h] = O_unnorm^T * 1/s  (s in psum row 64)
                    r = r_pool.tile([1, N], f32, name="r", tag="r")
                    for j in range(NJ):
                        nc.vector.reciprocal(
                            r[0:1, 512 * j:512 * (j + 1)], o_ps[j][64:65, :])
                    rb = rb_pool.tile([64, N], f32, name="rb", tag="rb")
                    nc.gpsimd.partition_broadcast(rb[:], r[0:1, :])
                    for j in range(NJ):
                        nc.vector.tensor_mul(
                            oT[h // 2][po:po + 64, 512 * j:512 * (j + 1)],
                            o_ps[j][0:64, :],
                            rb[0:64, 512 * j:512 * (j + 1)],
                        )

            # ---- phase 3: outT = w_proj^T @ oT + b_proj -----------------
            with (
                tc.tile_pool(name="p_ps", bufs=2, space="PSUM") as p_pool,
                tc.tile_pool(name="outc", bufs=2) as out_pool,
            ):
                for c in range(KT):
                    ps = p_pool.tile([128, N], f32, name="p_ps", tag="p_ps")
                    for j in range(NJ):
                        for m in range(KT):
                            nc.tensor.matmul(
                                ps[:, 512 * j:512 * (j + 1)],
                                lhsT=wp[m][:, 128 * c:128 * (c + 1)],
                                rhs=oT[m][:, 512 * j:512 * (j + 1)],
                                start=(m == 0), stop=(m == KT - 1),
                            )
                    oc = out_pool.tile([128, N], f32, name="oc", tag="oc")
                    nc.vector.tensor_scalar_add(oc[:], ps[:], ball[:, 12 + c:13 + c])
                    nc.sync.dma_start(out_e[128 * c:128 * (c + 1), :], oc[:])

    nc.compile()
    return nc


def prep_inputs(x, W_qkv, b_qkv, W_proj, b_proj):
    """Host-side shard + layout prep. Returns in_maps for 8 cores."""
    x = np.asarray(x, dtype=np.float32)
    W_qkv = np.asarray(W_qkv, dtype=np.float32)
    b_qkv = np.asarray(b_qkv, dtype=np.float32)
    W_proj = np.asarray(W_proj, dtype=np.float32)
    b_proj = np.asarray(b_proj, dtype=np.float32)

    w_qk = np.concatenate([W_qkv[:, :C] * SCALE, W_qkv[:, C:2 * C]], axis=1)
    w_qk = np.ascontiguousarray(w_qk).astype(BF16)
    w_v = np.ascontiguousarray(W_qkv[:, 2 * C:]).astype(BF16)
    w_p = W_proj.astype(BF16)

    b_qk = np.concatenate([b_qkv[:C] * SCALE, b_qkv[C:2 * C]])
    b_all = np.empty((128, 18), np.float32)
    b_all[:, :12] = b_qk.reshape(12, 128).T
    b_all[:, 12:] = b_proj.reshape(6, 128).T
    b_v = np.ascontiguousarray(
        np.broadcast_to(b_qkv[2 * C:].reshape(H, D), (128, H, D))).astype(np.float32)

    shared = {"w_qk": w_qk, "w_v": w_v, "w_proj": w_p, "b_all": b_all, "b_v": b_v}
    in_maps = []
    for b in range(NCORES):
        xT = np.ascontiguousarray(x[b].T).astype(BF16)
        m = dict(shared)
        m["xT"] = xT
        in_maps.append(m)
    return in_maps


def kernel(x, W_qkv, b_qkv, W_proj, b_proj):
    from concourse.bass_utils import run_bass_kernel_spmd

    nc = _CACHE.get("nc")
    if nc is None:
        nc = _CACHE["nc"] = build_nc()

    in_maps = prep_inputs(x, W_qkv, b_qkv, W_proj, b_proj)
    res = run_bass_kernel_spmd(nc, in_maps, core_ids=list(range(NCORES)))
    out = np.empty((B, N, C), np.float32)
    for b in range(NCORES):
        out[b] = res.results[b]["outT"].T
    return out
